# revision 33
# baseline (speedup 1.0000x reference)
"""Multi-head causal attention on 8 Trainium2 NeuronCores.

Problem (full shapes): x [4, 2048, 1024], wq/wk/wv [16, 1024, 64],
w_proj [1024, 1024], b_proj [1024] -> out [4, 2048, 1024].

Strategy (ONE SPMD launch, no collectives): batch x head-group sharding.
Core c owns batch (c % 4) and head half (c // 4): 8 heads of one batch.
Each core computes QKV + causal attention for its 8 heads and the
row-parallel partial projection y_part = w_proj[its 512 rows, :].T @
att_part for its batch. The host sums the two fp32 partials per batch
(the all-reduce of the sharding hint, done host-side since it is only
4 x 8 MB) and adds the bias. Every core carries exactly 1/8 of the
total MACs (6.45 G) and the whole thing is a single NEFF dispatch --
the per-launch relay floor dwarfs everything else wall-clock-wise.

Per core: x^T for its batch stays resident in SBUF (loaded in 512-token
chunks so the first QKV matmuls start early). For each of 4 head pairs:
QKV projections contract over C on the partition dim; scores are
computed transposed (scoresT[s, tq] = kT.T @ qT) so the exp'd weights
land directly in the [s, tq] layout the PE needs as the stationary
operand of wei @ v; causal block skipping on both the scores and the
wei@v matmuls. The softmax denominator comes free from a ones-column
appended to V (row 64 of the wei@v accumulation = sum of weights). Exp
on the scalar engine reading PSUM directly; diagonal-block masking on
the (otherwise idle) Pool engine; normalization on the vector engine.
Emission is software-pipelined: scores(i+1) issues before wei@v(i), the
softmax finalization (reciprocal broadcast + copy + normalize) is
deferred one iteration, and the V PE-transposes are deferred one chunk,
so the tensor engine rarely waits on the activation engine. The
attention output (channel-major [512, T]) stays in SBUF; the partial
projection streams yT [1024, T] fp32 to DRAM, alternating its PSUM
pools and copy engines. The Q/K projections run in fp8-e4m3 DoubleRow
mode (2 k-tiles per pass, 1.4x effective PE throughput and half the
matmul count): the host boosts wq/wk by 64 so e4m3 stays in its normal
range, and the combined 1/(64*64*sqrt(C)) rescale is folded into the
exp via the activation's scale operand. The softmax compresses the fp8
quantization (~0.8% total rel err vs the 2% gate). V, wei@v and the
projection stay bf16 (fp8 there would inject ~3% directly).
Accumulation is fp32 in PSUM. Measured ~0.41 ms device time per
execution; the single-launch wall time sits at the ~60 ms axon relay
floor.

kernel() is self-contained: hardcodes shapes, shards on host, runs the
single SPMD NEFF on cores 0-7, reassembles the full output on host.
"""

import numpy as np
import ml_dtypes

import concourse.bass as bass
import concourse.mybir as mybir
import concourse.tile as tile
from concourse.bass_utils import run_bass_kernel_spmd

B, T, C, H, D = 4, 2048, 1024, 16, 64
NCORES = 8
BT = B * T                 # 8192
CB = C // 128              # 8 contraction blocks over C
NB = T // 128              # 16 key blocks per batch
NPAIR = 4                  # head pairs per core (8 heads)
CC = 128 * NPAIR           # 512 channels owned per core
CBP = CC // 128            # 4 proj contraction blocks
F32 = mybir.dt.float32
FP8 = mybir.dt.float8e4
BF16 = mybir.dt.bfloat16
EXP = mybir.ActivationFunctionType.Exp
BF = ml_dtypes.bfloat16
E4 = ml_dtypes.float8_e4m3fn

_CACHE: dict = {}

# build-time tuning knobs (A/B'd on hardware)
TUNE = {"mask_eng": "gpsimd", "av_lag": 0, "proj_psum_dma": False, "wait_budget": 1,
        "pbcast_fin": False, "fp8_qk": True, "qkv_bufs": 2, "dma_vt": False, "fp8_av": False, "y_bf16": False, "wei_bufs": 8, "vt_bufs": 3, "exp_split": False, "sp_bufs": 4}


def split_waits(nc, budget=1):
    """Walrus codegen rejects instructions carrying too many semaphore
    waits; offload excess waits onto preceding same-engine NOPs."""
    k = 0
    for bb in nc.main_func.blocks:
        insts = bb.instructions
        i = 0
        while i < len(insts):
            ins = insts[i]
            si = getattr(ins, "sync_info", None)
            if si is not None and si.on_wait and len(si.on_wait) > budget:
                waits = list(si.on_wait)
                extra, keep = waits[:-budget], waits[-budget:]
                pos = i
                for c in range(0, len(extra), budget):
                    nop = mybir.InstNoOp(
                        name=f"I-waitsplit{k}",
                        engine=ins.engine,
                        ins=[],
                        outs=[],
                        sync_info=mybir.SyncInfo(
                            on_wait=extra[c : c + budget], on_update=[]
                        ),
                        bass_nofuse=True,
                    )
                    k += 1
                    insts.insert(pos, nop)
                    pos += 1
                    i += 1
                ins.sync_info = mybir.SyncInfo(
                    on_wait=keep, on_update=list(si.on_update or [])
                )
            i += 1
    return k


def _build(split=True, reps=1, phases=("qkv", "attn", "proj")):
    nc = bass.Bass()

    xT = nc.dram_tensor("xT", [C, T], BF16, kind="ExternalInput")
    if TUNE["fp8_qk"]:
        xT8 = nc.dram_tensor("xT8", [C, T], FP8, kind="ExternalInput")
        wq2 = nc.dram_tensor("wq8", [C, CC], FP8, kind="ExternalInput")
        wk2 = nc.dram_tensor("wk8", [C, CC], FP8, kind="ExternalInput")
    else:
        wq2 = nc.dram_tensor("wq2", [C, CC], BF16, kind="ExternalInput")
        wk2 = nc.dram_tensor("wk2", [C, CC], BF16, kind="ExternalInput")
    wv2 = nc.dram_tensor("wv2", [C, CC], BF16, kind="ExternalInput")
    wp = nc.dram_tensor("wp", [CC, C], BF16, kind="ExternalInput")
    YDT = BF16 if TUNE["y_bf16"] else F32
    yT = nc.dram_tensor("yT", [C, T], YDT, kind="ExternalOutput")

    ident_d = nc.inline_tensor(np.eye(128, dtype=BF), name="ident")
    # mask[s, tq] = 1 where s <= tq (keep); applied to the diagonal block
    mask_d = nc.inline_tensor(
        np.triu(np.ones((128, 128), dtype=BF)), name="mask"
    )
    ones_d = nc.inline_tensor(np.ones((1, 64), dtype=BF), name="ones64")
    mask8_d = nc.inline_tensor(
        np.triu(np.ones((128, 128), dtype=E4)), name="mask8"
    )

    with tile.TileContext(nc) as tc:
        with (
            tc.tile_pool(name="wpool", bufs=1) as wpool,
            tc.tile_pool(name="qkv", bufs=TUNE["qkv_bufs"]) as qkv_pool,
            tc.tile_pool(name="vt", bufs=TUNE["vt_bufs"]) as vt_pool,
            tc.tile_pool(name="wei", bufs=TUNE["wei_bufs"]) as wei_pool,
            tc.tile_pool(name="small", bufs=TUNE["sp_bufs"]) as spool,
            tc.tile_pool(name="outp", bufs=3) as opool,
            tc.tile_pool(name="ps_qkv", bufs=2, space="PSUM") as ps_qkv,
            tc.tile_pool(name="ps_sc", bufs=2, space="PSUM") as ps_sc,
            tc.tile_pool(name="ps_av", bufs=2, space="PSUM") as ps_av,
        ):
            QKDT = FP8 if TUNE["fp8_qk"] else BF16
            wq_sb = wpool.tile([128, CB, CC], QKDT)
            wk_sb = wpool.tile([128, CB, CC], QKDT)
            if TUNE["fp8_qk"]:
                xT8_sb = [
                    wpool.tile([128, CB, 512], FP8, name=f"xT8{i}")
                    for i in range(4)
                ]
            wv_sb = wpool.tile([128, CB, CC], BF16)
            wp_sb = wpool.tile([128, CBP, C], BF16)
            xT_sb = [wpool.tile([128, CB, 512], BF16, name=f"xT{i}") for i in range(4)]
            att_sb = wpool.tile([128, CBP, T], BF16)
            ident = wpool.tile([128, 128], BF16)
            mask = wpool.tile([128, 128], BF16)
            mask8 = wpool.tile([128, 128], FP8)
            ones_sb = wpool.tile([1, 64], BF16)

            rep_ctx = tc.For_i(0, reps, 1) if reps > 1 else None
            if rep_ctx is not None:
                rep_ctx.__enter__()

            # Input DMA inside the rep loop so reps-differenced timing
            # charges the full kernel including its HBM input traffic.
            nc.sync.dma_start(wq_sb[:], wq2[:].rearrange("(b p) m -> p b m", p=128))
            if TUNE["fp8_qk"]:
                for xch in range(4):
                    x0 = 512 * xch
                    nc.sync.dma_start(
                        xT8_sb[xch][:],
                        xT8[:, x0 : x0 + 512].rearrange("(b p) t -> p b t", p=128),
                    )
            for xch in range(4):
                x0 = 512 * xch
                nc.sync.dma_start(
                    xT_sb[xch][:],
                    xT[:, x0 : x0 + 512].rearrange("(b p) t -> p b t", p=128),
                )
            nc.sync.dma_start(wk_sb[:], wk2[:].rearrange("(b p) m -> p b m", p=128))
            nc.sync.dma_start(wv_sb[:], wv2[:].rearrange("(b p) m -> p b m", p=128))
            nc.sync.dma_start(ident[:], ident_d[:])
            nc.sync.dma_start(mask[:], mask_d[:])
            nc.sync.dma_start(mask8[:], mask8_d[:])
            nc.sync.dma_start(ones_sb[:], ones_d[:])
            nc.sync.dma_start(wp_sb[:], wp[:].rearrange("(b p) m -> p b m", p=128))

            # Software-pipelined emission: PE never waits on the ACT exp of
            # the block it just scored — scores(i+1) is issued before
            # wei@v(i), and the softmax finalization (reciprocal broadcast +
            # copy + normalize) is deferred one iteration so its PE matmul
            # lands after independent PE work. Same math, reordered issue.
            deferred = []  # (av_tile, recip_tile, att_sb slice)

            def flush_deferred():
                while deferred:
                    avt, r, ot_sl = deferred.pop(0)
                    if TUNE["pbcast_fin"]:
                        # broadcast 1/denominator across partitions on the
                        # Pool engine instead of a PE matmul
                        rb = spool.tile([64, 512], BF16, tag="rb", bufs=3)
                        nc.gpsimd.partition_broadcast(rb[:], r[:])
                        nc.vector.tensor_copy(ot_sl, avt[0:64, :])
                        nc.vector.tensor_mul(ot_sl, ot_sl, rb[:])
                    else:
                        # broadcast 1/denominator across partitions 64..127
                        # of the same PSUM bank via a K=1 ones matmul.
                        nc.tensor.matmul(
                            avt[64:128, :], ones_sb[:], r[:],
                            start=True, stop=True,
                        )
                        nc.vector.tensor_copy(ot_sl, avt[0:64, :])
                        nc.vector.tensor_mul(ot_sl, ot_sl, avt[64:128, :])

            pending_v = []  # (vt_tile, sb0) awaiting PE transpose into v2

            for p in range(NPAIR):
                cp = 128 * p  # channel offset of this head pair
                # ---- QKV for head pair p ----
                qT = qkv_pool.tile([128, T], BF16, tag="qT")
                kT = qkv_pool.tile([128, T], BF16, tag="kT")
                V2DT = FP8 if TUNE["fp8_av"] else BF16
                V2W = 128 if TUNE["fp8_av"] else 65
                v2 = qkv_pool.tile([128, NB, 2 * V2W], V2DT, tag="v2")
                # ones column at d=64 of each head slot (denominator row)
                nc.gpsimd.memset(v2[:, :, 64:65], 1.0)
                nc.gpsimd.memset(v2[:, :, V2W + 64 : V2W + 65], 1.0)
                if V2W > 65:
                    # zero the alignment padding (junk PSUM rows, never read)
                    nc.gpsimd.memset(v2[:, :, 65:V2W], 0.0)
                    nc.gpsimd.memset(v2[:, :, V2W + 65 : 2 * V2W], 0.0)

                def flush_pending_v(v2=v2):
                    while pending_v:
                        vt, sb0 = pending_v.pop(0)
                        if TUNE["dma_vt"]:
                            # XBAR DMA transpose ([d, t] -> [t, d]) on the
                            # idle DMA engines instead of PE matmul transposes
                            v2t = vt_pool.tile(
                                [128, 4, 128], BF16, tag="v2t", bufs=3
                            )
                            nc.sync.dma_start_transpose(v2t[:], vt[:])
                            nc.vector.tensor_copy(
                                v2[:, sb0 : sb0 + 4, 0:64], v2t[:, :, 0:64]
                            )
                            nc.vector.tensor_copy(
                                v2[:, sb0 : sb0 + 4, V2W : V2W + 64],
                                v2t[:, :, 64:128],
                            )
                        else:
                            pst = ps_qkv.tile([128, 512], BF16, tag="ps_qkv")
                            for tb in range(4):
                                nc.tensor.transpose(
                                    pst[:, 128 * tb : 128 * (tb + 1)],
                                    vt[:, 128 * tb : 128 * (tb + 1)],
                                    ident[:],
                                )
                            pst3 = pst[:].rearrange("p (t d) -> p t d", d=128)
                            nc.vector.tensor_copy(
                                v2[:, sb0 : sb0 + 4, 0:64], pst3[:, :, 0:64]
                            )
                            nc.vector.tensor_copy(
                                v2[:, sb0 : sb0 + 4, V2W : V2W + 64],
                                pst3[:, :, 64:128],
                            )

                for tch in range(T // 512):
                    tc0 = 512 * tch
                    for w_sb, dst in ((wq_sb, qT), (wk_sb, kT)):
                        ps = ps_qkv.tile([128, 512], F32, tag="ps_qkv")
                        if TUNE["fp8_qk"]:
                            for cb2 in range(CB // 2):
                                nc.tensor.matmul(
                                    ps[:],
                                    w_sb[:, 2 * cb2 : 2 * cb2 + 2, cp : cp + 128],
                                    xT8_sb[tch][:, 2 * cb2 : 2 * cb2 + 2, :],
                                    start=(cb2 == 0),
                                    stop=(cb2 == CB // 2 - 1),
                                    perf_mode=mybir.MatmulPerfMode.DoubleRow,
                                )
                        else:
                            for cb in range(CB):
                                nc.tensor.matmul(
                                    ps[:],
                                    w_sb[:, cb, cp : cp + 128],
                                    xT_sb[tch][:, cb, :],
                                    start=(cb == 0),
                                    stop=(cb == CB - 1),
                                )
                        nc.vector.tensor_copy(dst[:, tc0 : tc0 + 512], ps[:])
                    # v (both heads packed): vT now; the PE transpose into
                    # the [t, d] layout is deferred one chunk so the PSUM
                    # copy latency hides under the next chunk's matmuls.
                    ps = ps_qkv.tile([128, 512], F32, tag="ps_qkv")
                    for cb in range(CB):
                        nc.tensor.matmul(
                            ps[:],
                            wv_sb[:, cb, cp : cp + 128],
                            xT_sb[tch][:, cb, :],
                            start=(cb == 0),
                            stop=(cb == CB - 1),
                        )
                    vt = vt_pool.tile([128, 512], BF16, tag="vt")
                    nc.vector.tensor_copy(vt[:], ps[:])
                    pending_v.append((vt, 4 * tch))
                    if tch > 0:
                        flush_pending_v()

                if "attn" not in phases:
                    continue
                # ---- attention for head pair p ----
                def emit_scores(h, jh, i, sc_store, qT=qT, kT=kT):
                    hp = 64 * h
                    ts_ = max(1024 * jh, 128 * i)
                    w = 1024 * jh + 1024 - ts_
                    ps = ps_sc.tile([128, 1024], F32, tag="sc")
                    off = 0
                    while off < w:
                        n = min(512, w - off)
                        nc.tensor.matmul(
                            ps[:, off : off + n],
                            kT[hp : hp + 64, 128 * i : 128 * (i + 1)],
                            qT[hp : hp + 64, ts_ + off : ts_ + off + n],
                            start=True,
                            stop=True,
                        )
                        off += n
                    sc_store[i] = ps

                def make_av(h, jh, i, av, wt, v2=v2, V2W=V2W):
                    def emit_av():
                        hp = 64 * h
                        ts_ = max(1024 * jh, 128 * i)
                        nlast = 8 * jh + 7
                        for jj in range(2):
                            j = 2 * jh + jj
                            glo = max(512 * j, ts_)
                            ghi = 512 * j + 512
                            n = ghi - glo
                            if n <= 0:
                                continue
                            i_last = min(4 * j + 3, nlast)
                            nc.tensor.matmul(
                                av[j][0:65, glo - 512 * j : glo - 512 * j + n],
                                v2[:, i, V2W * h : V2W * h + 65],
                                wt[:, glo - ts_ : glo - ts_ + n],
                                start=(i == 0),
                                stop=(i == i_last),
                            )
                            if i == i_last:
                                r = spool.tile([1, 512], BF16, tag="recip")
                                with nc.allow_low_precision(
                                    reason="softmax recip in bf16"
                                ):
                                    nc.vector.reciprocal(r[:], av[j][64:65, :])
                                ot_sl = att_sb[
                                    hp : hp + 64, p, 512 * j : 512 * (j + 1)
                                ]
                                deferred.append((av[j], r, ot_sl))
                    return emit_av

                esc = 2.0 ** -17 if TUNE["fp8_qk"] else 1.0
                meng = getattr(nc, TUNE["mask_eng"])

                def make_av_dr(m, ts0, wt2, av, h, jh, nlast, v2=v2, p=p, V2W=V2W):
                    # fp8 DoubleRow wei@v: one matmul covers key blocks
                    # 2m and 2m+1 (the two free-dim k-tiles of wt2/v2).
                    def emit_av():
                        hp = 64 * h
                        for jj in range(2):
                            j = 2 * jh + jj
                            glo = max(512 * j, ts0)
                            ghi = 512 * j + 512
                            n = ghi - glo
                            if n <= 0:
                                continue
                            m_last = min(4 * j + 3, nlast) // 2
                            nc.tensor.matmul(
                                av[j][0:128, glo - 512 * j : glo - 512 * j + n],
                                v2[:, 2 * m : 2 * m + 2,
                                   V2W * h : V2W * h + 128],
                                wt2[:, :, glo - ts0 : glo - ts0 + n],
                                start=(m == 0),
                                stop=(m == m_last),
                                perf_mode=mybir.MatmulPerfMode.DoubleRow,
                            )
                            if m == m_last:
                                r = spool.tile([1, 512], BF16, tag="recip")
                                with nc.allow_low_precision(
                                    reason="softmax recip in bf16"
                                ):
                                    nc.vector.reciprocal(r[:], av[j][64:65, :])
                                ot_sl = att_sb[
                                    hp : hp + 64, p, 512 * j : 512 * (j + 1)
                                ]
                                deferred.append((av[j], r, ot_sl))
                    return emit_av

                pending_av = None
                for h in range(2):
                    hp = 64 * h
                    for jh in range(2):
                        av = {}
                        for jj in range(2):
                            j = 2 * jh + jj
                            av[j] = ps_av.tile(
                                [128, 512], F32, tag="av", name=f"av{j}"
                            )
                        sc_store = {}
                        nlast = 8 * jh + 7
                        if TUNE["fp8_av"]:
                            emit_scores(h, jh, 0, sc_store)
                            emit_scores(h, jh, 1, sc_store)
                            if h == 0 and jh == 0:
                                flush_pending_v()
                            for m in range((nlast + 1) // 2):
                                i0, i1 = 2 * m, 2 * m + 1
                                ts0 = max(1024 * jh, 128 * i0)
                                ts1 = max(1024 * jh, 128 * i1)
                                te = 1024 * jh + 1024
                                w0, w1 = te - ts0, te - ts1
                                off1 = ts1 - ts0  # 0 or 128
                                wt2 = wei_pool.tile(
                                    [128, 2, 1024], FP8, tag="wei"
                                )
                                ps0 = sc_store.pop(i0)
                                with nc.allow_low_precision(
                                    reason="softmax weights in fp8"
                                ):
                                    nc.scalar.activation(
                                        wt2[:, 0, 0:w0], ps0[:, :w0],
                                        EXP, scale=esc,
                                    )
                                if 128 * i0 >= 1024 * jh:
                                    meng.tensor_mul(
                                        wt2[:, 0, 0:128], wt2[:, 0, 0:128],
                                        mask8[:],
                                    )
                                if i0 + 2 <= nlast:
                                    emit_scores(h, jh, i0 + 2, sc_store)
                                if off1 > 0:
                                    nc.gpsimd.memset(wt2[:, 1, 0:off1], 0.0)
                                ps1 = sc_store.pop(i1)
                                with nc.allow_low_precision(
                                    reason="softmax weights in fp8"
                                ):
                                    nc.scalar.activation(
                                        wt2[:, 1, off1 : off1 + w1],
                                        ps1[:, :w1], EXP, scale=esc,
                                    )
                                if 128 * i1 >= 1024 * jh:
                                    meng.tensor_mul(
                                        wt2[:, 1, off1 : off1 + 128],
                                        wt2[:, 1, off1 : off1 + 128],
                                        mask8[:],
                                    )
                                if i1 + 2 <= nlast:
                                    emit_scores(h, jh, i1 + 2, sc_store)
                                flush_deferred()
                                if pending_av is not None:
                                    pending_av()
                                pending_av = make_av_dr(
                                    m, ts0, wt2, av, h, jh, nlast
                                )
                            continue
                        emit_scores(h, jh, 0, sc_store)
                        if h == 0 and jh == 0:
                            flush_pending_v()
                        for i in range(nlast + 1):
                            ts_ = max(1024 * jh, 128 * i)
                            w = 1024 * jh + 1024 - ts_
                            ps = sc_store.pop(i)
                            wt = wei_pool.tile([128, 1024], BF16, tag="wei")
                            if TUNE["exp_split"] and w > 512:
                                # halve exp latency before the first wei@v:
                                # the av matmuls can start once their column
                                # range is exp'd (subrange dependency)
                                nc.scalar.activation(
                                    wt[:, 0:512], ps[:, 0:512], EXP, scale=esc
                                )
                                nc.scalar.activation(
                                    wt[:, 512:w], ps[:, 512:w], EXP, scale=esc
                                )
                            else:
                                nc.scalar.activation(
                                    wt[:, :w], ps[:, :w], EXP, scale=esc
                                )
                            if 128 * i >= 1024 * jh:
                                # first 128 cols are the diagonal block
                                meng.tensor_mul(
                                    wt[:, 0:128], wt[:, 0:128], mask[:]
                                )
                            if i < nlast:
                                emit_scores(h, jh, i + 1, sc_store)
                            flush_deferred()
                            if TUNE["av_lag"]:
                                if pending_av is not None:
                                    pending_av()
                                pending_av = make_av(h, jh, i, av, wt)
                            else:
                                make_av(h, jh, i, av, wt)()
                # pair epilogue: last av row
                if pending_av is not None:
                    pending_av()

            flush_deferred()

            if "proj" not in phases:
                att_prev = att_sb  # keep name alive
            # ---- partial output projection: yT = wp_half.T @ att_half ----
            for mb in (range(CB) if "proj" in phases else []):
                for tch in range(T // 512):
                    k = (mb * (T // 512) + tch) % 2
                    if k == 0:
                        ps = ps_qkv.tile([128, 512], F32, tag="ps_qkv")
                    else:
                        ps = ps_av.tile([128, 512], F32, tag="av")
                    for cb in range(CBP):
                        nc.tensor.matmul(
                            ps[:],
                            wp_sb[:, cb, 128 * mb : 128 * (mb + 1)],
                            att_sb[:, cb, 512 * tch : 512 * (tch + 1)],
                            start=(cb == 0),
                            stop=(cb == CBP - 1),
                        )
                    ysl = yT[
                        128 * mb : 128 * (mb + 1),
                        512 * tch : 512 * (tch + 1),
                    ]
                    if TUNE["proj_psum_dma"]:
                        nc.sync.dma_start(ysl, ps[:])
                    else:
                        ot = opool.tile([128, 512], YDT, tag="ot", bufs=4)
                        with nc.allow_low_precision(
                            reason="proj partial rounded to bf16; host sums in fp32"
                        ):
                            if k == 0:
                                nc.scalar.copy(ot[:], ps[:])
                            else:
                                nc.vector.tensor_copy(ot[:], ps[:])
                        nc.sync.dma_start(ysl, ot[:])

            if rep_ctx is not None:
                rep_ctx.__exit__(None, None, None)

    if split:
        split_waits(nc, budget=TUNE["wait_budget"])
    return nc


def _get_nc():
    if "nc" not in _CACHE:
        _CACHE["nc"] = _build()
    return _CACHE["nc"]


def make_in_maps(x, wq, wk, wv, w_proj):
    x = np.asarray(x, np.float32)
    scale = np.float32(C) ** -0.5
    fp8 = TUNE["fp8_qk"]
    # [H, C, D] -> [C, H*D] head-major columns. bf16 mode folds the C^-0.5
    # scale into wq; fp8 mode instead boosts wq/wk by 64 (e4m3 normal range)
    # and compensates with scale=2^-17 at the exp.
    wqf = np.asarray(wq, np.float32).transpose(1, 0, 2).reshape(C, C)
    wkf = np.asarray(wk, np.float32).transpose(1, 0, 2).reshape(C, C)
    wq2 = np.ascontiguousarray(64.0 * wqf).astype(E4) if fp8 else \
        np.ascontiguousarray(wqf * scale).astype(BF)
    wk2 = np.ascontiguousarray(64.0 * wkf).astype(E4) if fp8 else \
        np.ascontiguousarray(wkf).astype(BF)
    wv2 = np.ascontiguousarray(
        np.asarray(wv, np.float32).transpose(1, 0, 2).reshape(C, C)
    ).astype(BF)
    wpb = np.asarray(w_proj, np.float32).astype(BF)
    xTs = [np.ascontiguousarray(x[b].T).astype(BF) for b in range(B)]
    xT8s = [xt.astype(E4) for xt in xTs] if fp8 else None
    maps = []
    for c in range(NCORES):
        b, hh = c % B, c // B
        cs = CC * hh  # this core's channel slice
        m = {
            "xT": xTs[b],
            "wv2": np.ascontiguousarray(wv2[:, cs : cs + CC]),
            "wp": np.ascontiguousarray(wpb[cs : cs + CC, :]),
        }
        if fp8:
            m["xT8"] = xT8s[b]
            m["wq8"] = np.ascontiguousarray(wq2[:, cs : cs + CC])
            m["wk8"] = np.ascontiguousarray(wk2[:, cs : cs + CC])
        else:
            m["wq2"] = np.ascontiguousarray(wq2[:, cs : cs + CC])
            m["wk2"] = np.ascontiguousarray(wk2[:, cs : cs + CC])
        maps.append(m)
    return maps


def assemble_output(results, b_proj):
    y = np.empty((B, T, C), dtype=np.float32)
    for b in range(B):
        # sum the two head-half projection partials (host all-reduce)
        y[b] = (
            np.asarray(results[b]["yT"], np.float32)
            + np.asarray(results[b + B]["yT"], np.float32)
        ).T
    y += np.asarray(b_proj, np.float32)
    return y


def kernel(x, wq, wk, wv, w_proj, b_proj):
    res = run_bass_kernel_spmd(
        _get_nc(),
        make_in_maps(x, wq, wk, wv, w_proj),
        core_ids=list(range(NCORES)),
    )
    return assemble_output(res.results, b_proj)


# revision 35
# speedup vs baseline: 1.7340x; 1.7340x over previous
"""Multi-head causal attention on 8 Trainium2 NeuronCores.

Problem (full shapes): x [4, 2048, 1024], wq/wk/wv [16, 1024, 64],
w_proj [1024, 1024], b_proj [1024] -> out [4, 2048, 1024].

Strategy (ONE SPMD launch, no collectives): batch x head-group sharding.
Core c owns batch (c % 4) and head half (c // 4): 8 heads of one batch.
Each core computes QKV + causal attention for its 8 heads and the
row-parallel partial projection y_part = w_proj[its 512 rows, :].T @
att_part for its batch. The host sums the two fp32 partials per batch
(the all-reduce of the sharding hint, done host-side since it is only
4 x 8 MB) and adds the bias. Every core carries exactly 1/8 of the
total MACs (6.45 G) and the whole thing is a single NEFF dispatch --
the per-launch relay floor dwarfs everything else wall-clock-wise.

Per core: x^T for its batch stays resident in SBUF (loaded in 512-token
chunks so the first QKV matmuls start early). For each of 4 head pairs:
QKV projections contract over C on the partition dim; scores are
computed transposed (scoresT[s, tq] = kT.T @ qT) so the exp'd weights
land directly in the [s, tq] layout the PE needs as the stationary
operand of wei @ v; causal block skipping on both the scores and the
wei@v matmuls. The softmax denominator comes free from a ones-column
appended to V (row 64 of the wei@v accumulation = sum of weights). Exp
on the scalar engine reading PSUM directly; diagonal-block masking on
the (otherwise idle) Pool engine; normalization on the vector engine.
Emission is software-pipelined: scores(i+1) issues before wei@v(i), the
softmax finalization (reciprocal broadcast + copy + normalize) is
deferred one iteration, and the V PE-transposes are deferred one chunk,
so the tensor engine rarely waits on the activation engine. The
attention output (channel-major [512, T]) stays in SBUF; the partial
projection streams yT [1024, T] fp32 to DRAM, alternating its PSUM
pools and copy engines. The Q/K projections run in fp8-e4m3 DoubleRow
mode (2 k-tiles per pass, 1.4x effective PE throughput and half the
matmul count): the host boosts wq/wk by 64 so e4m3 stays in its normal
range, and the combined 1/(64*64*sqrt(C)) rescale is folded into the
exp via the activation's scale operand. The softmax compresses the fp8
quantization (~0.8% total rel err vs the 2% gate). V, wei@v and the
projection stay bf16 (fp8 there would inject ~3% directly).
Accumulation is fp32 in PSUM. Measured ~0.41 ms device time per
execution; the single-launch wall time sits at the ~60 ms axon relay
floor.

kernel() is self-contained: hardcodes shapes, shards on host, runs the
single SPMD NEFF on cores 0-7, reassembles the full output on host.
"""

import numpy as np
import ml_dtypes

import concourse.bass as bass
import concourse.mybir as mybir
import concourse.tile as tile
from concourse.bass_utils import run_bass_kernel_spmd

B, T, C, H, D = 4, 2048, 1024, 16, 64
NCORES = 8
BT = B * T                 # 8192
CB = C // 128              # 8 contraction blocks over C
NB = T // 128              # 16 key blocks per batch
NPAIR = 4                  # head pairs per core (8 heads)
CC = 128 * NPAIR           # 512 channels owned per core
CBP = CC // 128            # 4 proj contraction blocks
F32 = mybir.dt.float32
FP8 = mybir.dt.float8e4
BF16 = mybir.dt.bfloat16
EXP = mybir.ActivationFunctionType.Exp
BF = ml_dtypes.bfloat16
E4 = ml_dtypes.float8_e4m3fn

_CACHE: dict = {}

# build-time tuning knobs (A/B'd on hardware)
TUNE = {"mask_eng": "gpsimd", "av_lag": 0, "proj_psum_dma": False, "wait_budget": 1,
        "pbcast_fin": False, "fp8_qk": True, "qkv_bufs": 2, "dma_vt": False, "fp8_av": False, "y_bf16": False, "wei_bufs": 8, "vt_bufs": 3, "exp_split": False, "sp_bufs": 4, "proj_pools": 2}


def split_waits(nc, budget=1):
    """Walrus codegen rejects instructions carrying too many semaphore
    waits; offload excess waits onto preceding same-engine NOPs."""
    k = 0
    for bb in nc.main_func.blocks:
        insts = bb.instructions
        i = 0
        while i < len(insts):
            ins = insts[i]
            si = getattr(ins, "sync_info", None)
            if si is not None and si.on_wait and len(si.on_wait) > budget:
                waits = list(si.on_wait)
                extra, keep = waits[:-budget], waits[-budget:]
                pos = i
                for c in range(0, len(extra), budget):
                    nop = mybir.InstNoOp(
                        name=f"I-waitsplit{k}",
                        engine=ins.engine,
                        ins=[],
                        outs=[],
                        sync_info=mybir.SyncInfo(
                            on_wait=extra[c : c + budget], on_update=[]
                        ),
                        bass_nofuse=True,
                    )
                    k += 1
                    insts.insert(pos, nop)
                    pos += 1
                    i += 1
                ins.sync_info = mybir.SyncInfo(
                    on_wait=keep, on_update=list(si.on_update or [])
                )
            i += 1
    return k


def _build(split=True, reps=1, phases=("qkv", "attn", "proj")):
    nc = bass.Bass()

    xT = nc.dram_tensor("xT", [C, T], BF16, kind="ExternalInput")
    if TUNE["fp8_qk"]:
        xT8 = nc.dram_tensor("xT8", [C, T], FP8, kind="ExternalInput")
        wq2 = nc.dram_tensor("wq8", [C, CC], FP8, kind="ExternalInput")
        wk2 = nc.dram_tensor("wk8", [C, CC], FP8, kind="ExternalInput")
    else:
        wq2 = nc.dram_tensor("wq2", [C, CC], BF16, kind="ExternalInput")
        wk2 = nc.dram_tensor("wk2", [C, CC], BF16, kind="ExternalInput")
    wv2 = nc.dram_tensor("wv2", [C, CC], BF16, kind="ExternalInput")
    wp = nc.dram_tensor("wp", [CC, C], BF16, kind="ExternalInput")
    YDT = BF16 if TUNE["y_bf16"] else F32
    yT = nc.dram_tensor("yT", [C, T], YDT, kind="ExternalOutput")

    ident_d = nc.inline_tensor(np.eye(128, dtype=BF), name="ident")
    # mask[s, tq] = 1 where s <= tq (keep); applied to the diagonal block
    mask_d = nc.inline_tensor(
        np.triu(np.ones((128, 128), dtype=BF)), name="mask"
    )
    ones_d = nc.inline_tensor(np.ones((1, 64), dtype=BF), name="ones64")
    mask8_d = nc.inline_tensor(
        np.triu(np.ones((128, 128), dtype=E4)), name="mask8"
    )

    with tile.TileContext(nc) as tc:
        with (
            tc.tile_pool(name="wpool", bufs=1) as wpool,
            tc.tile_pool(name="qkv", bufs=TUNE["qkv_bufs"]) as qkv_pool,
            tc.tile_pool(name="vt", bufs=TUNE["vt_bufs"]) as vt_pool,
            tc.tile_pool(name="wei", bufs=TUNE["wei_bufs"]) as wei_pool,
            tc.tile_pool(name="small", bufs=TUNE["sp_bufs"]) as spool,
            tc.tile_pool(name="outp", bufs=3) as opool,
            tc.tile_pool(name="ps_qkv", bufs=2, space="PSUM") as ps_qkv,
            tc.tile_pool(name="ps_sc", bufs=2, space="PSUM") as ps_sc,
            tc.tile_pool(name="ps_av", bufs=2, space="PSUM") as ps_av,
        ):
            QKDT = FP8 if TUNE["fp8_qk"] else BF16
            wq_sb = wpool.tile([128, CB, CC], QKDT)
            wk_sb = wpool.tile([128, CB, CC], QKDT)
            if TUNE["fp8_qk"]:
                xT8_sb = [
                    wpool.tile([128, CB, 512], FP8, name=f"xT8{i}")
                    for i in range(4)
                ]
            wv_sb = wpool.tile([128, CB, CC], BF16)
            wp_sb = wpool.tile([128, CBP, C], BF16)
            xT_sb = [wpool.tile([128, CB, 512], BF16, name=f"xT{i}") for i in range(4)]
            att_sb = wpool.tile([128, CBP, T], BF16)
            ident = wpool.tile([128, 128], BF16)
            mask = wpool.tile([128, 128], BF16)
            mask8 = wpool.tile([128, 128], FP8)
            ones_sb = wpool.tile([1, 64], BF16)

            rep_ctx = tc.For_i(0, reps, 1) if reps > 1 else None
            if rep_ctx is not None:
                rep_ctx.__enter__()

            # Input DMA inside the rep loop so reps-differenced timing
            # charges the full kernel including its HBM input traffic.
            nc.sync.dma_start(wq_sb[:], wq2[:].rearrange("(b p) m -> p b m", p=128))
            if TUNE["fp8_qk"]:
                for xch in range(4):
                    x0 = 512 * xch
                    nc.sync.dma_start(
                        xT8_sb[xch][:],
                        xT8[:, x0 : x0 + 512].rearrange("(b p) t -> p b t", p=128),
                    )
            for xch in range(4):
                x0 = 512 * xch
                nc.sync.dma_start(
                    xT_sb[xch][:],
                    xT[:, x0 : x0 + 512].rearrange("(b p) t -> p b t", p=128),
                )
            nc.sync.dma_start(wk_sb[:], wk2[:].rearrange("(b p) m -> p b m", p=128))
            nc.sync.dma_start(wv_sb[:], wv2[:].rearrange("(b p) m -> p b m", p=128))
            nc.sync.dma_start(ident[:], ident_d[:])
            nc.sync.dma_start(mask[:], mask_d[:])
            nc.sync.dma_start(mask8[:], mask8_d[:])
            nc.sync.dma_start(ones_sb[:], ones_d[:])
            nc.sync.dma_start(wp_sb[:], wp[:].rearrange("(b p) m -> p b m", p=128))

            # Software-pipelined emission: PE never waits on the ACT exp of
            # the block it just scored — scores(i+1) is issued before
            # wei@v(i), and the softmax finalization (reciprocal broadcast +
            # copy + normalize) is deferred one iteration so its PE matmul
            # lands after independent PE work. Same math, reordered issue.
            deferred = []  # (av_tile, recip_tile, att_sb slice)

            def flush_deferred():
                while deferred:
                    avt, r, ot_sl = deferred.pop(0)
                    if TUNE["pbcast_fin"]:
                        # broadcast 1/denominator across partitions on the
                        # Pool engine instead of a PE matmul
                        rb = spool.tile([64, 512], BF16, tag="rb", bufs=3)
                        nc.gpsimd.partition_broadcast(rb[:], r[:])
                        nc.vector.tensor_copy(ot_sl, avt[0:64, :])
                        nc.vector.tensor_mul(ot_sl, ot_sl, rb[:])
                    else:
                        # broadcast 1/denominator across partitions 64..127
                        # of the same PSUM bank via a K=1 ones matmul.
                        nc.tensor.matmul(
                            avt[64:128, :], ones_sb[:], r[:],
                            start=True, stop=True,
                        )
                        nc.vector.tensor_copy(ot_sl, avt[0:64, :])
                        nc.vector.tensor_mul(ot_sl, ot_sl, avt[64:128, :])

            pending_v = []  # (vt_tile, sb0) awaiting PE transpose into v2

            for p in range(NPAIR):
                cp = 128 * p  # channel offset of this head pair
                # ---- QKV for head pair p ----
                qT = qkv_pool.tile([128, T], BF16, tag="qT")
                kT = qkv_pool.tile([128, T], BF16, tag="kT")
                V2DT = FP8 if TUNE["fp8_av"] else BF16
                V2W = 128 if TUNE["fp8_av"] else 65
                v2 = qkv_pool.tile([128, NB, 2 * V2W], V2DT, tag="v2")
                # ones column at d=64 of each head slot (denominator row)
                nc.gpsimd.memset(v2[:, :, 64:65], 1.0)
                nc.gpsimd.memset(v2[:, :, V2W + 64 : V2W + 65], 1.0)
                if V2W > 65:
                    # zero the alignment padding (junk PSUM rows, never read)
                    nc.gpsimd.memset(v2[:, :, 65:V2W], 0.0)
                    nc.gpsimd.memset(v2[:, :, V2W + 65 : 2 * V2W], 0.0)

                def flush_pending_v(v2=v2):
                    while pending_v:
                        vt, sb0 = pending_v.pop(0)
                        if TUNE["dma_vt"]:
                            # XBAR DMA transpose ([d, t] -> [t, d]) on the
                            # idle DMA engines instead of PE matmul transposes
                            v2t = vt_pool.tile(
                                [128, 4, 128], BF16, tag="v2t", bufs=3
                            )
                            nc.sync.dma_start_transpose(v2t[:], vt[:])
                            nc.vector.tensor_copy(
                                v2[:, sb0 : sb0 + 4, 0:64], v2t[:, :, 0:64]
                            )
                            nc.vector.tensor_copy(
                                v2[:, sb0 : sb0 + 4, V2W : V2W + 64],
                                v2t[:, :, 64:128],
                            )
                        else:
                            pst = ps_qkv.tile([128, 512], BF16, tag="ps_qkv")
                            for tb in range(4):
                                nc.tensor.transpose(
                                    pst[:, 128 * tb : 128 * (tb + 1)],
                                    vt[:, 128 * tb : 128 * (tb + 1)],
                                    ident[:],
                                )
                            pst3 = pst[:].rearrange("p (t d) -> p t d", d=128)
                            nc.vector.tensor_copy(
                                v2[:, sb0 : sb0 + 4, 0:64], pst3[:, :, 0:64]
                            )
                            nc.vector.tensor_copy(
                                v2[:, sb0 : sb0 + 4, V2W : V2W + 64],
                                pst3[:, :, 64:128],
                            )

                for tch in range(T // 512):
                    tc0 = 512 * tch
                    for w_sb, dst in ((wq_sb, qT), (wk_sb, kT)):
                        ps = ps_qkv.tile([128, 512], F32, tag="ps_qkv")
                        if TUNE["fp8_qk"]:
                            for cb2 in range(CB // 2):
                                nc.tensor.matmul(
                                    ps[:],
                                    w_sb[:, 2 * cb2 : 2 * cb2 + 2, cp : cp + 128],
                                    xT8_sb[tch][:, 2 * cb2 : 2 * cb2 + 2, :],
                                    start=(cb2 == 0),
                                    stop=(cb2 == CB // 2 - 1),
                                    perf_mode=mybir.MatmulPerfMode.DoubleRow,
                                )
                        else:
                            for cb in range(CB):
                                nc.tensor.matmul(
                                    ps[:],
                                    w_sb[:, cb, cp : cp + 128],
                                    xT_sb[tch][:, cb, :],
                                    start=(cb == 0),
                                    stop=(cb == CB - 1),
                                )
                        nc.vector.tensor_copy(dst[:, tc0 : tc0 + 512], ps[:])
                    # v (both heads packed): vT now; the PE transpose into
                    # the [t, d] layout is deferred one chunk so the PSUM
                    # copy latency hides under the next chunk's matmuls.
                    ps = ps_qkv.tile([128, 512], F32, tag="ps_qkv")
                    for cb in range(CB):
                        nc.tensor.matmul(
                            ps[:],
                            wv_sb[:, cb, cp : cp + 128],
                            xT_sb[tch][:, cb, :],
                            start=(cb == 0),
                            stop=(cb == CB - 1),
                        )
                    vt = vt_pool.tile([128, 512], BF16, tag="vt")
                    nc.vector.tensor_copy(vt[:], ps[:])
                    pending_v.append((vt, 4 * tch))
                    if tch > 0:
                        flush_pending_v()

                if "attn" not in phases:
                    continue
                # ---- attention for head pair p ----
                def emit_scores(h, jh, i, sc_store, qT=qT, kT=kT):
                    hp = 64 * h
                    ts_ = max(1024 * jh, 128 * i)
                    w = 1024 * jh + 1024 - ts_
                    ps = ps_sc.tile([128, 1024], F32, tag="sc")
                    off = 0
                    while off < w:
                        n = min(512, w - off)
                        nc.tensor.matmul(
                            ps[:, off : off + n],
                            kT[hp : hp + 64, 128 * i : 128 * (i + 1)],
                            qT[hp : hp + 64, ts_ + off : ts_ + off + n],
                            start=True,
                            stop=True,
                        )
                        off += n
                    sc_store[i] = ps

                def make_av(h, jh, i, av, wt, v2=v2, V2W=V2W):
                    def emit_av():
                        hp = 64 * h
                        ts_ = max(1024 * jh, 128 * i)
                        nlast = 8 * jh + 7
                        for jj in range(2):
                            j = 2 * jh + jj
                            glo = max(512 * j, ts_)
                            ghi = 512 * j + 512
                            n = ghi - glo
                            if n <= 0:
                                continue
                            i_last = min(4 * j + 3, nlast)
                            nc.tensor.matmul(
                                av[j][0:65, glo - 512 * j : glo - 512 * j + n],
                                v2[:, i, V2W * h : V2W * h + 65],
                                wt[:, glo - ts_ : glo - ts_ + n],
                                start=(i == 0),
                                stop=(i == i_last),
                            )
                            if i == i_last:
                                r = spool.tile([1, 512], BF16, tag="recip")
                                with nc.allow_low_precision(
                                    reason="softmax recip in bf16"
                                ):
                                    nc.vector.reciprocal(r[:], av[j][64:65, :])
                                ot_sl = att_sb[
                                    hp : hp + 64, p, 512 * j : 512 * (j + 1)
                                ]
                                deferred.append((av[j], r, ot_sl))
                    return emit_av

                esc = 2.0 ** -17 if TUNE["fp8_qk"] else 1.0
                meng = getattr(nc, TUNE["mask_eng"])

                def make_av_dr(m, ts0, wt2, av, h, jh, nlast, v2=v2, p=p, V2W=V2W):
                    # fp8 DoubleRow wei@v: one matmul covers key blocks
                    # 2m and 2m+1 (the two free-dim k-tiles of wt2/v2).
                    def emit_av():
                        hp = 64 * h
                        for jj in range(2):
                            j = 2 * jh + jj
                            glo = max(512 * j, ts0)
                            ghi = 512 * j + 512
                            n = ghi - glo
                            if n <= 0:
                                continue
                            m_last = min(4 * j + 3, nlast) // 2
                            nc.tensor.matmul(
                                av[j][0:128, glo - 512 * j : glo - 512 * j + n],
                                v2[:, 2 * m : 2 * m + 2,
                                   V2W * h : V2W * h + 128],
                                wt2[:, :, glo - ts0 : glo - ts0 + n],
                                start=(m == 0),
                                stop=(m == m_last),
                                perf_mode=mybir.MatmulPerfMode.DoubleRow,
                            )
                            if m == m_last:
                                r = spool.tile([1, 512], BF16, tag="recip")
                                with nc.allow_low_precision(
                                    reason="softmax recip in bf16"
                                ):
                                    nc.vector.reciprocal(r[:], av[j][64:65, :])
                                ot_sl = att_sb[
                                    hp : hp + 64, p, 512 * j : 512 * (j + 1)
                                ]
                                deferred.append((av[j], r, ot_sl))
                    return emit_av

                pending_av = None
                for h in range(2):
                    hp = 64 * h
                    for jh in range(2):
                        av = {}
                        for jj in range(2):
                            j = 2 * jh + jj
                            av[j] = ps_av.tile(
                                [128, 512], F32, tag="av", name=f"av{j}"
                            )
                        sc_store = {}
                        nlast = 8 * jh + 7
                        if TUNE["fp8_av"]:
                            emit_scores(h, jh, 0, sc_store)
                            emit_scores(h, jh, 1, sc_store)
                            if h == 0 and jh == 0:
                                flush_pending_v()
                            for m in range((nlast + 1) // 2):
                                i0, i1 = 2 * m, 2 * m + 1
                                ts0 = max(1024 * jh, 128 * i0)
                                ts1 = max(1024 * jh, 128 * i1)
                                te = 1024 * jh + 1024
                                w0, w1 = te - ts0, te - ts1
                                off1 = ts1 - ts0  # 0 or 128
                                wt2 = wei_pool.tile(
                                    [128, 2, 1024], FP8, tag="wei"
                                )
                                ps0 = sc_store.pop(i0)
                                with nc.allow_low_precision(
                                    reason="softmax weights in fp8"
                                ):
                                    nc.scalar.activation(
                                        wt2[:, 0, 0:w0], ps0[:, :w0],
                                        EXP, scale=esc,
                                    )
                                if 128 * i0 >= 1024 * jh:
                                    meng.tensor_mul(
                                        wt2[:, 0, 0:128], wt2[:, 0, 0:128],
                                        mask8[:],
                                    )
                                if i0 + 2 <= nlast:
                                    emit_scores(h, jh, i0 + 2, sc_store)
                                if off1 > 0:
                                    nc.gpsimd.memset(wt2[:, 1, 0:off1], 0.0)
                                ps1 = sc_store.pop(i1)
                                with nc.allow_low_precision(
                                    reason="softmax weights in fp8"
                                ):
                                    nc.scalar.activation(
                                        wt2[:, 1, off1 : off1 + w1],
                                        ps1[:, :w1], EXP, scale=esc,
                                    )
                                if 128 * i1 >= 1024 * jh:
                                    meng.tensor_mul(
                                        wt2[:, 1, off1 : off1 + 128],
                                        wt2[:, 1, off1 : off1 + 128],
                                        mask8[:],
                                    )
                                if i1 + 2 <= nlast:
                                    emit_scores(h, jh, i1 + 2, sc_store)
                                flush_deferred()
                                if pending_av is not None:
                                    pending_av()
                                pending_av = make_av_dr(
                                    m, ts0, wt2, av, h, jh, nlast
                                )
                            continue
                        emit_scores(h, jh, 0, sc_store)
                        if h == 0 and jh == 0:
                            flush_pending_v()
                        for i in range(nlast + 1):
                            ts_ = max(1024 * jh, 128 * i)
                            w = 1024 * jh + 1024 - ts_
                            ps = sc_store.pop(i)
                            wt = wei_pool.tile([128, 1024], BF16, tag="wei")
                            if TUNE["exp_split"] and w > 512:
                                # halve exp latency before the first wei@v:
                                # the av matmuls can start once their column
                                # range is exp'd (subrange dependency)
                                nc.scalar.activation(
                                    wt[:, 0:512], ps[:, 0:512], EXP, scale=esc
                                )
                                nc.scalar.activation(
                                    wt[:, 512:w], ps[:, 512:w], EXP, scale=esc
                                )
                            else:
                                nc.scalar.activation(
                                    wt[:, :w], ps[:, :w], EXP, scale=esc
                                )
                            if 128 * i >= 1024 * jh:
                                # first 128 cols are the diagonal block
                                meng.tensor_mul(
                                    wt[:, 0:128], wt[:, 0:128], mask[:]
                                )
                            if i < nlast:
                                emit_scores(h, jh, i + 1, sc_store)
                            flush_deferred()
                            if TUNE["av_lag"]:
                                if pending_av is not None:
                                    pending_av()
                                pending_av = make_av(h, jh, i, av, wt)
                            else:
                                make_av(h, jh, i, av, wt)()
                # pair epilogue: last av row
                if pending_av is not None:
                    pending_av()

            flush_deferred()

            if "proj" not in phases:
                att_prev = att_sb  # keep name alive
            # ---- partial output projection: yT = wp_half.T @ att_half ----
            for mb in (range(CB) if "proj" in phases else []):
                for tch in range(T // 512):
                    k = (mb * (T // 512) + tch) % TUNE["proj_pools"]
                    if k == 0:
                        ps = ps_qkv.tile([128, 512], F32, tag="ps_qkv")
                    elif k == 1:
                        ps = ps_av.tile([128, 512], F32, tag="av")
                    else:
                        # the scores pool is idle during proj; use the first
                        # bank of its [128, 1024] tiles as a third rotation
                        ps_full = ps_sc.tile(
                            [128, 1024], F32, tag="sc", name="projsc"
                        )
                        ps = ps_full[:, 0:512]
                    for cb in range(CBP):
                        nc.tensor.matmul(
                            ps[:],
                            wp_sb[:, cb, 128 * mb : 128 * (mb + 1)],
                            att_sb[:, cb, 512 * tch : 512 * (tch + 1)],
                            start=(cb == 0),
                            stop=(cb == CBP - 1),
                        )
                    ysl = yT[
                        128 * mb : 128 * (mb + 1),
                        512 * tch : 512 * (tch + 1),
                    ]
                    if TUNE["proj_psum_dma"]:
                        nc.sync.dma_start(ysl, ps[:])
                    else:
                        ot = opool.tile([128, 512], YDT, tag="ot", bufs=4)
                        with nc.allow_low_precision(
                            reason="proj partial rounded to bf16; host sums in fp32"
                        ):
                            if k % 2 == 0:
                                nc.scalar.copy(ot[:], ps[:])
                            else:
                                nc.vector.tensor_copy(ot[:], ps[:])
                        nc.sync.dma_start(ysl, ot[:])

            if rep_ctx is not None:
                rep_ctx.__exit__(None, None, None)

    if split:
        split_waits(nc, budget=TUNE["wait_budget"])
    return nc


def _get_nc():
    if "nc" not in _CACHE:
        _CACHE["nc"] = _build()
    return _CACHE["nc"]


def make_in_maps(x, wq, wk, wv, w_proj):
    x = np.asarray(x, np.float32)
    scale = np.float32(C) ** -0.5
    fp8 = TUNE["fp8_qk"]
    # [H, C, D] -> [C, H*D] head-major columns. bf16 mode folds the C^-0.5
    # scale into wq; fp8 mode instead boosts wq/wk by 64 (e4m3 normal range)
    # and compensates with scale=2^-17 at the exp.
    wqf = np.asarray(wq, np.float32).transpose(1, 0, 2).reshape(C, C)
    wkf = np.asarray(wk, np.float32).transpose(1, 0, 2).reshape(C, C)
    wq2 = np.ascontiguousarray(64.0 * wqf).astype(E4) if fp8 else \
        np.ascontiguousarray(wqf * scale).astype(BF)
    wk2 = np.ascontiguousarray(64.0 * wkf).astype(E4) if fp8 else \
        np.ascontiguousarray(wkf).astype(BF)
    wv2 = np.ascontiguousarray(
        np.asarray(wv, np.float32).transpose(1, 0, 2).reshape(C, C)
    ).astype(BF)
    wpb = np.asarray(w_proj, np.float32).astype(BF)
    xTs = [np.ascontiguousarray(x[b].T).astype(BF) for b in range(B)]
    xT8s = [xt.astype(E4) for xt in xTs] if fp8 else None
    maps = []
    for c in range(NCORES):
        b, hh = c % B, c // B
        cs = CC * hh  # this core's channel slice
        m = {
            "xT": xTs[b],
            "wv2": np.ascontiguousarray(wv2[:, cs : cs + CC]),
            "wp": np.ascontiguousarray(wpb[cs : cs + CC, :]),
        }
        if fp8:
            m["xT8"] = xT8s[b]
            m["wq8"] = np.ascontiguousarray(wq2[:, cs : cs + CC])
            m["wk8"] = np.ascontiguousarray(wk2[:, cs : cs + CC])
        else:
            m["wq2"] = np.ascontiguousarray(wq2[:, cs : cs + CC])
            m["wk2"] = np.ascontiguousarray(wk2[:, cs : cs + CC])
        maps.append(m)
    return maps


def assemble_output(results, b_proj):
    y = np.empty((B, T, C), dtype=np.float32)
    for b in range(B):
        # sum the two head-half projection partials (host all-reduce)
        y[b] = (
            np.asarray(results[b]["yT"], np.float32)
            + np.asarray(results[b + B]["yT"], np.float32)
        ).T
    y += np.asarray(b_proj, np.float32)
    return y


def kernel(x, wq, wk, wv, w_proj, b_proj):
    res = run_bass_kernel_spmd(
        _get_nc(),
        make_in_maps(x, wq, wk, wv, w_proj),
        core_ids=list(range(NCORES)),
    )
    return assemble_output(res.results, b_proj)
